# revision 41
# baseline (speedup 1.0000x reference)
"""Paged-KV varlen causal GQA attention for Trainium2, sharded over 8 NeuronCores.

Problem (hardcoded from spec): T=4096 tokens, 16 q heads / 8 kv heads, head_dim=64,
scale=0.125. k/v are scattered into paged caches via slot_mapping, read back, and
causal varlen attention (segments from cu_seqlens) is computed.

Sharding: tensor-parallel over kv heads -- core h gets kv head h and its 2 GQA
query heads. slot_mapping / cu_seqlens handled on host (index math only).

Device kernel (per core), per causal segment:
  sT[keys, queries] = kT.T @ qT        (PE; both q heads concurrently via row tiling, K=64)
  w = exp(0.125 * sT)                  (ScalarE, PSUM->SBUF bf16; no max-subtraction --
                                        scores are O(5), exp stays in fp32 range)
  diagonal 128-tiles: w *= upper-tri mask (VectorE)
  oT[65, q] += [v | 1].T @ w           (PE accumulate; row 64 = softmax denominator)
  o[q, 0:64] = oT.T / oT.T[:, 64]      (PE transpose + VectorE reciprocal/scale)
"""

import os
from contextlib import ExitStack
from math import ceil

import numpy as np
import ml_dtypes

import concourse.bass as bass
import concourse.mybir as mybir
import concourse.tile as tile
from concourse import bacc
from concourse.bass_utils import run_bass_kernel_spmd
from concourse.masks import make_identity

NKV = 8
G = 2
D = 64
SCALE = 0.125

# test.py pokes these for profiling
TRACE = bool(int(os.environ.get("KERNEL_TRACE", "0")))
LAST_RESULT = None

_PROGRAM_CACHE = {}


def _build_program(T, segments):
    f32 = mybir.dt.float32
    bf16 = mybir.dt.bfloat16

    nc = bacc.Bacc(
        "TRN2",
        target_bir_lowering=False,
        debug=False,
        enable_asserts=False,
        num_devices=8,
    )
    qT_d = nc.dram_tensor("qT", [128, T], bf16, kind="ExternalInput").ap()
    kT_d = nc.dram_tensor("kT", [64, T], bf16, kind="ExternalInput").ap()
    v_d = nc.dram_tensor("v", [T, D], bf16, kind="ExternalInput").ap()
    o_d = nc.dram_tensor("o", [T, 2 * D], f32, kind="ExternalOutput").ap()

    with tile.TileContext(nc) as tc, ExitStack() as ctx:
        const = ctx.enter_context(tc.tile_pool(name="const", bufs=1))
        qkpool = ctx.enter_context(tc.tile_pool(name="qk", bufs=1))
        vpool = ctx.enter_context(tc.tile_pool(name="vt", bufs=4))
        spool = ctx.enter_context(tc.tile_pool(name="se", bufs=6))
        opool = ctx.enter_context(tc.tile_pool(name="osb", bufs=4))
        ps_o = ctx.enter_context(tc.tile_pool(name="ps_o", bufs=1, space="PSUM"))
        ps_s = ctx.enter_context(tc.tile_pool(name="ps_s", bufs=3, space="PSUM"))

        ident = const.tile([128, 128], f32)
        make_identity(nc, ident)
        # trimask[p, c] = 1 if c >= p else 0 (valid = query col >= key partition)
        trimask = const.tile([128, 128], bf16)
        nc.gpsimd.memset(trimask, 0.0)
        nc.gpsimd.affine_select(
            out=trimask,
            in_=trimask,
            compare_op=mybir.AluOpType.is_gt,
            fill=1.0,
            base=0,
            pattern=[[-1, 128]],
            channel_multiplier=1,
        )

        qT = qkpool.tile([128, T], bf16)
        # kT duplicated on partitions 0-63 and 64-127 so the two q heads'
        # QK matmuls can run in different PE row-groups concurrently.
        kT = qkpool.tile([128, T], bf16)

        def finalize_range(s0, q0, h, m, b0, c_lo, c_hi, t65, tp_tag, act_copy, uniq):
            """Transpose + normalize t65 cols [c_lo, c_hi) (128-aligned c_lo) and
            DMA the rows out. t65 already holds the PSUM copy for that range."""
            nchunk = ceil((c_hi - c_lo) / 128)
            tp = ps_o.tile(
                [128, 65 * nchunk], f32, tag=tp_tag, name=f"tp_{uniq}"
            )
            for j in range(nchunk):
                n = min(128, c_hi - c_lo - j * 128)
                nc.tensor.transpose(
                    tp[:n, 65 * j : 65 * j + 65],
                    t65[:65, c_lo + j * 128 : c_lo + j * 128 + n],
                    ident[:65, :65],
                )
            rcp = opool.tile([128, nchunk], f32, tag="rcp", name=f"rcp_{uniq}")
            osb = opool.tile([128, D * nchunk], f32, tag="osb", name=f"osb_{uniq}")
            nf = (c_hi - c_lo) // 128  # chunks with all 128 rows written
            if nf:
                nc.vector.reciprocal(rcp[:, :nf], tp[:, D : 65 * nf : 65])
                tp_v = tp[:, : 65 * nf].rearrange("p (c k) -> p c k", k=65)[:, :, 0:D]
                rcp_v, tp_v = bass.broadcast_tensor_aps(
                    rcp[:, :nf].rearrange("p (c k) -> p c k", k=1), tp_v
                )
                nc.vector.tensor_mul(
                    osb[:, : D * nf].rearrange("p (c k) -> p c k", k=D), tp_v, rcp_v
                )
            if nf < nchunk:
                # partial last chunk: only n rows of tp were written -- use
                # exact-row ops so nothing stale is read
                n = (c_hi - c_lo) - nf * 128
                nc.vector.reciprocal(
                    rcp[:n, nf : nf + 1], tp[:n, 65 * nf + D : 65 * nf + D + 1]
                )
                nc.vector.tensor_scalar_mul(
                    osb[:n, D * nf : D * nf + D],
                    tp[:n, 65 * nf : 65 * nf + D],
                    rcp[:n, nf : nf + 1],
                )
            # output DMAs go out on the DVE DGE queue -- separate HWDGE
            # generation pipe from the SP queue that carries all input DMAs
            r0 = s0 + q0 + b0 + c_lo
            span = c_hi - c_lo
            cfull = span // 128
            if cfull:
                nc.sync.dma_start(
                    o_d[r0 : r0 + cfull * 128, D * h : D * h + D].rearrange(
                        "(c p) k -> p c k", p=128
                    ),
                    osb.rearrange("p (c k) -> p c k", k=D)[:, :cfull, :],
                )
            if span % 128:
                n = span % 128
                nc.sync.dma_start(
                    o_d[r0 + cfull * 128 : r0 + span, D * h : D * h + D],
                    osb[:n, D * cfull : D * cfull + D],
                )

        def finalize_block(s0, q0, h, m, b0, blen, oT, act_copy=False):
            """Transpose + normalize one [65, blen] oT block and DMA it out."""
            uniq = f"{s0}_{q0}_{h}_{m}"
            t65 = opool.tile([65, 512], f32, tag="t65", name=f"t65_{uniq}")
            if act_copy:
                nc.scalar.copy(t65[:, :blen], oT)
            else:
                nc.vector.tensor_copy(t65[:, :blen], oT)
            finalize_range(
                s0, q0, h, m, b0, 0, blen, t65, f"oT_{m}", act_copy, uniq
            )

        supers = []
        for (s0, s1) in segments:
            L = s1 - s0
            for q0 in range(0, L, 1024):
                supers.append((s0, s1, q0, min(1024, L - q0)))

        seen_seg = set()
        pending_final = None
        for si, (s0, s1, q0, qlen) in enumerate(supers):
            is_last_super = si == len(supers) - 1
            if s0 not in seen_seg:
                seen_seg.add(s0)
                if si == 0:
                    # first segment: order + split DMAs so head-0's first QK
                    # unblocks as early as possible
                    nc.sync.dma_start(kT[0:64, s0:s1], kT_d[:, s0:s1])
                    nc.sync.dma_start(qT[0:64, s0:s1], qT_d[0:64, s0:s1])
                    nc.sync.dma_start(kT[64:128, s0:s1], kT_d[:, s0:s1])
                    nc.sync.dma_start(qT[64:128, s0:s1], qT_d[64:128, s0:s1])
                else:
                    nc.sync.dma_start(qT[:, s0:s1], qT_d[:, s0:s1])
                    nc.sync.dma_start(kT[0:64, s0:s1], kT_d[:, s0:s1])
                    nc.sync.dma_start(kT[64:128, s0:s1], kT_d[:, s0:s1])
            edges = list(range(0, qlen, 512)) + [qlen]
            nblk = len(edges) - 1
            kend = q0 + qlen
            nkt = ceil(kend / 128)
            # stage all v tiles for this super-block in one DMA; col 64 of
            # each [128, 65] tile is the ones column for the denominator row
            vst = vpool.tile([128, nkt, D + 1], bf16, tag="vt")
            nfull = kend // 128
            if nfull:
                nc.sync.dma_start(
                    vst[:, :nfull, 0:D],
                    v_d[s0 : s0 + nfull * 128, :].rearrange("(n p) d -> p n d", p=128),
                )
            if kend % 128:
                rem = kend % 128
                nc.sync.dma_start(
                    vst[:rem, nfull, 0:D], v_d[s0 + nfull * 128 : s0 + kend, :]
                )
            nc.any.memset(vst[:, :, D : D + 1], 1.0)
            # heads processed sequentially: halves the live oT accumulators
            # (2 PSUM banks instead of 4) so the score pool gets 3 slots --
            # letting PE run further ahead of the ScalarE exp bottleneck
            for h in range(G):
                oT_ps = [
                    ps_o.tile(
                        [65, edges[m + 1] - edges[m]],
                        f32,
                        tag=f"oT_{m}",
                        name=f"oT_{s0}_{q0}_{h}_{m}",
                    )
                    for m in range(nblk)
                ]
                for kt in range(nkt):
                    klo = kt * 128
                    kp = min(128, kend - klo)
                    vt = vst[:, kt, :]
                    c0 = max(q0, klo)  # first attending query col in this super
                    span = kend - c0
                    # score tile anchored on the 512-block grid so each QK write
                    # [a0, b1) stays inside one PSUM bank (matmul outputs must
                    # not cross bank boundaries). Spans within the last 512-block
                    # use a 1-bank tile from a separate tag so the 2-bank slots
                    # are already free for the next pass's first QKs.
                    small = (c0 - q0) >= edges[nblk - 1]
                    anchor = q0 + (edges[nblk - 1] if small else 0)
                    if small:
                        sp = ps_s.tile([128, 512], f32, tag="sps", bufs=2)
                    else:
                        sp = ps_s.tile([128, 1024], f32, tag="spb", bufs=2)
                    for m in range(nblk):
                        b0 = q0 + edges[m]
                        b1 = q0 + edges[m + 1]
                        a0 = max(b0, c0)
                        if a0 >= b1:
                            continue
                        nc.tensor.matmul(
                            sp[:kp, a0 - anchor : b1 - anchor],
                            kT[64 * h : 64 * h + 64, s0 + klo : s0 + klo + kp],
                            qT[64 * h : 64 * h + 64, s0 + a0 : s0 + b1],
                            start=True,
                            stop=True,
                            tile_position=(64 * h, 0),
                        )
                    if kt == 0 and pending_final is not None:
                        # deferred finalize of the previous pass's last block:
                        # emitted after this pass's first QK so PE feeds the
                        # ScalarE exp pipeline before the finalize transposes
                        finalize_block(*pending_final)
                        pending_final = None
                    se = spool.tile([128, 1024], bf16, tag="se")
                    if si == 0 and h == 0 and kt == 0:
                        # kernel warmup: exp per block so the first exp only
                        # waits on the first QK matmul
                        for m in range(nblk):
                            e0 = max(q0 + edges[m], c0) - c0
                            e1 = q0 + edges[m + 1] - c0
                            if e0 >= e1:
                                continue
                            nc.scalar.activation(
                                se[:kp, e0:e1],
                                sp[:kp, c0 - anchor + e0 : c0 - anchor + e1],
                                mybir.ActivationFunctionType.Exp,
                                scale=SCALE,
                            )
                            if m == 0 and c0 == klo:
                                dn = min(kp, span)
                                nc.vector.tensor_mul(
                                    se[:kp, :dn], se[:kp, :dn], trimask[:kp, :dn]
                                )
                    else:
                        nc.scalar.activation(
                            se[:kp, :span],
                            sp[:kp, c0 - anchor : c0 - anchor + span],
                            mybir.ActivationFunctionType.Exp,
                            scale=SCALE,
                        )
                        if c0 == klo:
                            # diagonal tile: queries [klo, klo+kp) get the mask
                            dn = min(kp, span)
                            nc.vector.tensor_mul(
                                se[:kp, :dn], se[:kp, :dn], trimask[:kp, :dn]
                            )
                    for m in range(nblk):
                        b0 = q0 + edges[m]
                        b1 = q0 + edges[m + 1]
                        a0 = max(b0, c0)
                        if a0 >= b1:
                            continue
                        blen = edges[m + 1] - edges[m]
                        last = kt == ceil(b1 / 128) - 1
                        # the very last output block is finalized in two stages
                        # so only a small copy/normalize/DMA chain remains after
                        # the final PV lands
                        tail2 = (
                            is_last_super and h == G - 1 and m == nblk - 1 and nkt >= 2
                        )
                        c_split = 128 * (nkt - 1) - (q0 + edges[m])
                        split2 = tail2 and 128 <= c_split < blen
                        stage1_here = split2 and kt == nkt - 2
                        nc.tensor.matmul(
                            oT_ps[m][:, a0 - b0 : b1 - b0],
                            vt[:kp, : D + 1],
                            se[:kp, a0 - c0 : b1 - c0],
                            start=(kt == 0),
                            # close the accumulation group early for the split
                            # block so stage 1 may read its final columns
                            stop=last or stage1_here,
                            skip_group_check=(split2 and kt == nkt - 1),
                        )
                        if stage1_here:
                            uniq = f"{s0}_{q0}_{h}_{m}_s1"
                            t65s = opool.tile(
                                [65, 512], f32, tag="t65", name=f"t65_{uniq}"
                            )
                            nc.vector.tensor_copy(
                                t65s[:, :c_split], oT_ps[m][:, :c_split]
                            )
                            other = m - 1 if m >= 1 else m + 1
                            finalize_range(
                                s0, q0, h, m, edges[m], 0, c_split,
                                t65s, f"oT_{other}", False, uniq,
                            )
                            tail_t65 = t65s
                        if last:
                            if split2:
                                uniq = f"{s0}_{q0}_{h}_{m}_s2"
                                nc.scalar.copy(
                                    tail_t65[:, c_split:blen],
                                    oT_ps[m][:, c_split:blen],
                                )
                                finalize_range(
                                    s0, q0, h, m, edges[m], c_split, blen,
                                    tail_t65, f"oT_{m}", True, uniq,
                                )
                                continue
                            args = (s0, q0, h, m, edges[m], blen, oT_ps[m])
                            if kt == nkt - 1:
                                # end-of-pass block: defer past the next pass's
                                # first QK so PE keeps feeding the ACT pipeline
                                pending_final = args
                            else:
                                # emit finalize right away; the scheduler
                                # overlaps it with the remaining kt iterations
                                finalize_block(*args)

        if pending_final is not None:
            finalize_block(*pending_final, act_copy=True)
            pending_final = None

    nc.compile()
    return nc


def _segments_from_cu(cu_seqlens, T):
    edges = sorted(set([0, T] + [int(c) for c in cu_seqlens if 0 < int(c) < T]))
    return [(edges[i], edges[i + 1]) for i in range(len(edges) - 1)]


def kernel(q, k, v, k_cache, v_cache, slot_mapping, cu_seqlens):
    global LAST_RESULT
    T = q.shape[0]
    nslots = k_cache.shape[0]

    # Emulate scatter-then-gather through the paged cache: for duplicate slots
    # the last writer wins, so token i reads back k[lastw[slot[i]]].
    slot = np.asarray(slot_mapping, dtype=np.int64)
    lastw = np.zeros(nslots, dtype=np.int64)
    lastw[slot] = np.arange(T)
    lw = lastw[slot]
    k_eff = np.asarray(k)[lw]
    v_eff = np.asarray(v)[lw]

    segments = _segments_from_cu(np.asarray(cu_seqlens), T)
    key = (T, tuple(segments))
    if key not in _PROGRAM_CACHE:
        _PROGRAM_CACHE[key] = _build_program(T, segments)
    nc = _PROGRAM_CACHE[key]

    bf = ml_dtypes.bfloat16
    qh = np.ascontiguousarray(
        np.asarray(q).reshape(T, NKV * G, D).transpose(1, 2, 0)
    ).astype(bf)  # [16, 64, T]
    kh = np.ascontiguousarray(k_eff.reshape(T, NKV, D).transpose(1, 2, 0)).astype(bf)
    vh = v_eff.reshape(T, NKV, D).astype(bf)  # [T, 8, 64]

    in_maps = [
        {
            "qT": np.ascontiguousarray(qh[2 * h : 2 * h + 2].reshape(128, T)),
            "kT": np.ascontiguousarray(kh[h]),
            "v": np.ascontiguousarray(vh[:, h, :]),
        }
        for h in range(NKV)
    ]

    res = run_bass_kernel_spmd(nc, in_maps, core_ids=list(range(8)), trace=TRACE)
    LAST_RESULT = res

    out = np.empty((T, NKV * G * D), dtype=np.float32)
    ov = out.reshape(T, NKV, G * D)
    for h in range(NKV):
        ov[:, h, :] = res.results[h]["o"]
    return out


# revision 46
# speedup vs baseline: 1.0283x; 1.0283x over previous
"""Paged-KV varlen causal GQA attention for Trainium2, sharded over 8 NeuronCores.

Problem (hardcoded from spec): T=4096 tokens, 16 q heads / 8 kv heads, head_dim=64,
scale=0.125. k/v are scattered into paged caches via slot_mapping, read back, and
causal varlen attention (segments from cu_seqlens) is computed.

Sharding: tensor-parallel over kv heads -- core h gets kv head h and its 2 GQA
query heads. slot_mapping / cu_seqlens handled on host (index math only).

Device kernel (per core), per causal segment:
  sT[keys, queries] = kT.T @ qT        (PE; both q heads concurrently via row tiling, K=64)
  w = exp(0.125 * sT)                  (ScalarE, PSUM->SBUF bf16; no max-subtraction --
                                        scores are O(5), exp stays in fp32 range)
  diagonal 128-tiles: w *= upper-tri mask (VectorE)
  oT[65, q] += [v | 1].T @ w           (PE accumulate; row 64 = softmax denominator)
  o[q, 0:64] = oT.T / oT.T[:, 64]      (PE transpose + VectorE reciprocal/scale)
"""

import os
from contextlib import ExitStack
from math import ceil

import numpy as np
import ml_dtypes

import concourse.bass as bass
import concourse.mybir as mybir
import concourse.tile as tile
from concourse import bacc
from concourse.bass_utils import run_bass_kernel_spmd
from concourse.masks import make_identity

NKV = 8
G = 2
D = 64
SCALE = 0.125

# test.py pokes these for profiling
TRACE = bool(int(os.environ.get("KERNEL_TRACE", "0")))
LAST_RESULT = None

_PROGRAM_CACHE = {}


def _build_program(T, segments):
    f32 = mybir.dt.float32
    bf16 = mybir.dt.bfloat16

    nc = bacc.Bacc(
        "TRN2",
        target_bir_lowering=False,
        debug=False,
        enable_asserts=False,
        num_devices=8,
    )
    qT_d = nc.dram_tensor("qT", [128, T], bf16, kind="ExternalInput").ap()
    kT_d = nc.dram_tensor("kT", [64, T], bf16, kind="ExternalInput").ap()
    v_d = nc.dram_tensor("v", [T, D], bf16, kind="ExternalInput").ap()
    o_d = nc.dram_tensor("o", [T, 2 * D], f32, kind="ExternalOutput").ap()

    with tile.TileContext(nc) as tc, ExitStack() as ctx:
        const = ctx.enter_context(tc.tile_pool(name="const", bufs=1))
        qkpool = ctx.enter_context(tc.tile_pool(name="qk", bufs=1))
        vpool = ctx.enter_context(tc.tile_pool(name="vt", bufs=4))
        spool = ctx.enter_context(tc.tile_pool(name="se", bufs=6))
        opool = ctx.enter_context(tc.tile_pool(name="osb", bufs=4))
        ps_o = ctx.enter_context(tc.tile_pool(name="ps_o", bufs=1, space="PSUM"))
        ps_s = ctx.enter_context(tc.tile_pool(name="ps_s", bufs=3, space="PSUM"))

        ident = const.tile([128, 128], f32)
        make_identity(nc, ident)
        # trimask[p, c] = 1 if c >= p else 0 (valid = query col >= key partition)
        trimask = const.tile([128, 128], bf16)
        nc.gpsimd.memset(trimask, 0.0)
        nc.gpsimd.affine_select(
            out=trimask,
            in_=trimask,
            compare_op=mybir.AluOpType.is_gt,
            fill=1.0,
            base=0,
            pattern=[[-1, 128]],
            channel_multiplier=1,
        )

        qT = qkpool.tile([128, T], bf16)
        # kT duplicated on partitions 0-63 and 64-127 so the two q heads'
        # QK matmuls can run in different PE row-groups concurrently.
        kT = qkpool.tile([128, T], bf16)

        def finalize_range(s0, q0, h, m, b0, c_lo, c_hi, t65, tp_tag, act_copy, uniq):
            """Transpose + normalize t65 cols [c_lo, c_hi) (128-aligned c_lo) and
            DMA the rows out. t65 already holds the PSUM copy for that range."""
            nchunk = ceil((c_hi - c_lo) / 128)
            tp = ps_o.tile(
                [128, 65 * nchunk], f32, tag=tp_tag, name=f"tp_{uniq}"
            )
            for j in range(nchunk):
                n = min(128, c_hi - c_lo - j * 128)
                nc.tensor.transpose(
                    tp[:n, 65 * j : 65 * j + 65],
                    t65[:65, c_lo + j * 128 : c_lo + j * 128 + n],
                    ident[:65, :65],
                )
            rcp = opool.tile([128, nchunk], f32, tag="rcp", name=f"rcp_{uniq}")
            osb = opool.tile([128, D * nchunk], f32, tag="osb", name=f"osb_{uniq}")
            nf = (c_hi - c_lo) // 128  # chunks with all 128 rows written
            if nf:
                nc.vector.reciprocal(rcp[:, :nf], tp[:, D : 65 * nf : 65])
                tp_v = tp[:, : 65 * nf].rearrange("p (c k) -> p c k", k=65)[:, :, 0:D]
                rcp_v, tp_v = bass.broadcast_tensor_aps(
                    rcp[:, :nf].rearrange("p (c k) -> p c k", k=1), tp_v
                )
                nc.vector.tensor_mul(
                    osb[:, : D * nf].rearrange("p (c k) -> p c k", k=D), tp_v, rcp_v
                )
            if nf < nchunk:
                # partial last chunk: only n rows of tp were written -- use
                # exact-row ops so nothing stale is read
                n = (c_hi - c_lo) - nf * 128
                nc.vector.reciprocal(
                    rcp[:n, nf : nf + 1], tp[:n, 65 * nf + D : 65 * nf + D + 1]
                )
                nc.vector.tensor_scalar_mul(
                    osb[:n, D * nf : D * nf + D],
                    tp[:n, 65 * nf : 65 * nf + D],
                    rcp[:n, nf : nf + 1],
                )
            # output DMAs go out on the DVE DGE queue -- separate HWDGE
            # generation pipe from the SP queue that carries all input DMAs
            r0 = s0 + q0 + b0 + c_lo
            span = c_hi - c_lo
            cfull = span // 128
            if cfull:
                nc.sync.dma_start(
                    o_d[r0 : r0 + cfull * 128, D * h : D * h + D].rearrange(
                        "(c p) k -> p c k", p=128
                    ),
                    osb.rearrange("p (c k) -> p c k", k=D)[:, :cfull, :],
                )
            if span % 128:
                n = span % 128
                nc.sync.dma_start(
                    o_d[r0 + cfull * 128 : r0 + span, D * h : D * h + D],
                    osb[:n, D * cfull : D * cfull + D],
                )

        def finalize_block(s0, q0, h, m, b0, blen, oT, act_copy=False):
            """Transpose + normalize one [65, blen] oT block and DMA it out."""
            uniq = f"{s0}_{q0}_{h}_{m}"
            t65 = opool.tile([65, 512], f32, tag="t65", name=f"t65_{uniq}")
            if act_copy:
                nc.scalar.copy(t65[:, :blen], oT)
            else:
                nc.vector.tensor_copy(t65[:, :blen], oT)
            finalize_range(
                s0, q0, h, m, b0, 0, blen, t65, f"oT_{m}", act_copy, uniq
            )

        supers = []
        for (s0, s1) in segments:
            L = s1 - s0
            for q0 in range(0, L, 1024):
                supers.append((s0, s1, q0, min(1024, L - q0)))

        seen_seg = set()
        pending_final = None
        for si, (s0, s1, q0, qlen) in enumerate(supers):
            is_last_super = si == len(supers) - 1
            if s0 not in seen_seg:
                seen_seg.add(s0)
                if si == 0:
                    # first segment: order + split DMAs so head-0's first QK
                    # unblocks as early as possible
                    nc.sync.dma_start(kT[0:64, s0:s1], kT_d[:, s0:s1])
                    nc.sync.dma_start(qT[0:64, s0:s1], qT_d[0:64, s0:s1])
                    nc.sync.dma_start(kT[64:128, s0:s1], kT_d[:, s0:s1])
                    nc.sync.dma_start(qT[64:128, s0:s1], qT_d[64:128, s0:s1])
                else:
                    nc.sync.dma_start(qT[:, s0:s1], qT_d[:, s0:s1])
                    nc.sync.dma_start(kT[0:64, s0:s1], kT_d[:, s0:s1])
                    nc.sync.dma_start(kT[64:128, s0:s1], kT_d[:, s0:s1])
            edges = list(range(0, qlen, 512)) + [qlen]
            nblk = len(edges) - 1
            kend = q0 + qlen
            nkt = ceil(kend / 128)
            # stage all v tiles for this super-block in one DMA; col 64 of
            # each [128, 65] tile is the ones column for the denominator row
            vst = vpool.tile([128, nkt, D + 1], bf16, tag="vt")
            nfull = kend // 128
            if nfull:
                nc.sync.dma_start(
                    vst[:, :nfull, 0:D],
                    v_d[s0 : s0 + nfull * 128, :].rearrange("(n p) d -> p n d", p=128),
                )
            if kend % 128:
                rem = kend % 128
                nc.sync.dma_start(
                    vst[:rem, nfull, 0:D], v_d[s0 + nfull * 128 : s0 + kend, :]
                )
            nc.any.memset(vst[:, :, D : D + 1], 1.0)
            # heads processed sequentially: halves the live oT accumulators
            # (2 PSUM banks instead of 4) so the score pool gets 3 slots --
            # letting PE run further ahead of the ScalarE exp bottleneck
            for h in range(G):
                oT_ps = [
                    ps_o.tile(
                        [65, edges[m + 1] - edges[m]],
                        f32,
                        tag=f"oT_{m}",
                        name=f"oT_{s0}_{q0}_{h}_{m}",
                    )
                    for m in range(nblk)
                ]
                def kt_info(kt):
                    klo = kt * 128
                    kp = min(128, kend - klo)
                    c0 = max(q0, klo)
                    return klo, kp, c0, kend - c0

                # pack kts into score-tile groups: big kts (span beyond the
                # last 512-block) stay singletons in the 2-bank "spb" rotation;
                # consecutive full small kts merge into one tile (and one exp)
                # as long as each member's span stays inside a PSUM bank
                groups = []  # list[list[(kt, off)]] -- small groups first-fit
                small_groups = []
                for kt in range(nkt):
                    klo, kp, c0, span = kt_info(kt)
                    small = (c0 - q0) >= edges[nblk - 1]
                    if not small:
                        groups.append([(kt, 0)])
                        continue
                    placed = False
                    # the final pass keeps strict kt order (consecutive merges
                    # only) so the two-stage tail finalize stays valid
                    allow_ff = not (is_last_super and h == G - 1)
                    if kp == 128:
                        for g in small_groups:
                            if kt_info(g[0][0])[1] != 128:
                                continue
                            if not allow_ff and (
                                g is not small_groups[-1] or g[-1][0] != kt - 1
                            ):
                                continue
                            off = g[-1][1] + kt_info(g[-1][0])[3]
                            if (off % 512) + span <= 512 and off + span <= 512:
                                g.append((kt, off))
                                placed = True
                                break
                    if not placed:
                        g = [(kt, 0)]
                        small_groups.append(g)
                        groups.append(g)

                # last EMITTED kt that writes each block (emission order can
                # differ from kt order after first-fit packing) -- keys the
                # accumulation-group stop flag and finalize triggers
                last_emit = {}
                for g in groups:
                    for kt, _ in g:
                        klo = kt * 128
                        c0 = max(q0, klo)
                        for m in range(nblk):
                            if max(q0 + edges[m], c0) < q0 + edges[m + 1]:
                                last_emit[m] = kt
                n_small = 0
                for gi, members in enumerate(groups):
                    g_kt0 = members[0][0]
                    g_klo0, g_kp0, g_c00, g_span0 = kt_info(g_kt0)
                    g_small = (g_c00 - q0) >= edges[nblk - 1]
                    if g_small:
                        lk, loff = members[-1]
                        total = loff + kt_info(lk)[3]
                        sp = ps_s.tile([128, 512], f32, tag="sps", bufs=2)
                        n_small += 1
                        rows = 128 if len(members) > 1 else g_kp0
                    else:
                        sp = ps_s.tile([128, 1024], f32, tag="spb", bufs=2)
                        total = g_span0
                        rows = g_kp0
                    for kt, off in members:
                        klo, kp, c0, span = kt_info(kt)
                        for m in range(nblk):
                            b0 = q0 + edges[m]
                            b1 = q0 + edges[m + 1]
                            a0 = max(b0, c0)
                            if a0 >= b1:
                                continue
                            lo = (off + a0 - c0) if g_small else (a0 - q0)
                            nc.tensor.matmul(
                                sp[:kp, lo : lo + b1 - a0],
                                kT[64 * h : 64 * h + 64, s0 + klo : s0 + klo + kp],
                                qT[64 * h : 64 * h + 64, s0 + a0 : s0 + b1],
                                start=True,
                                stop=True,
                                tile_position=(64 * h, 0),
                            )
                    if g_kt0 == 0 and pending_final is not None:
                        # deferred finalize of the previous pass's last block:
                        # emitted after this pass's first QK so PE feeds the
                        # ScalarE exp pipeline before the finalize transposes
                        finalize_block(*pending_final)
                        pending_final = None
                    se = spool.tile([128, 1024], bf16, tag="se")
                    if si == 0 and h == 0 and g_kt0 == 0 and not g_small:
                        # kernel warmup: exp per block so the first exp only
                        # waits on the first QK matmul
                        for m in range(nblk):
                            e0 = max(q0 + edges[m], g_c00) - g_c00
                            e1 = q0 + edges[m + 1] - g_c00
                            if e0 >= e1:
                                continue
                            nc.scalar.activation(
                                se[:rows, e0:e1],
                                sp[:rows, g_c00 - q0 + e0 : g_c00 - q0 + e1],
                                mybir.ActivationFunctionType.Exp,
                                scale=SCALE,
                            )
                            if m == 0 and g_c00 == g_klo0:
                                dn = min(g_kp0, g_span0)
                                nc.vector.tensor_mul(
                                    se[:rows, :dn], se[:rows, :dn], trimask[:rows, :dn]
                                )
                    else:
                        elo = 0 if g_small else g_c00 - q0
                        nc.scalar.activation(
                            se[:rows, :total],
                            sp[:rows, elo : elo + total],
                            mybir.ActivationFunctionType.Exp,
                            scale=SCALE,
                        )
                        for kt, off in members:
                            klo, kp, c0, span = kt_info(kt)
                            if c0 == klo:
                                # diagonal tile: causal mask on this member
                                dn = min(kp, span)
                                so = off if g_small else 0
                                nc.vector.tensor_mul(
                                    se[:kp, so : so + dn],
                                    se[:kp, so : so + dn],
                                    trimask[:kp, :dn],
                                )
                    for kt, off in members:
                        klo, kp, c0, span = kt_info(kt)
                        for m in range(nblk):
                            b0 = q0 + edges[m]
                            b1 = q0 + edges[m + 1]
                            a0 = max(b0, c0)
                            if a0 >= b1:
                                continue
                            blen = edges[m + 1] - edges[m]
                            last = kt == last_emit[m]
                            tail2 = (
                                is_last_super
                                and h == G - 1
                                and m == nblk - 1
                                and nkt >= 2
                            )
                            c_split = 128 * (nkt - 1) - (q0 + edges[m])
                            split2 = tail2 and 128 <= c_split < blen
                            stage1_here = split2 and kt == nkt - 2
                            so = off if g_small else (c0 - g_c00 if False else 0)
                            ro = (off + a0 - c0) if g_small else (a0 - g_c00)
                            nc.tensor.matmul(
                                oT_ps[m][:, a0 - b0 : b1 - b0],
                                vst[:, kt, :][:kp, : D + 1],
                                se[:kp, ro : ro + b1 - a0],
                                start=(kt == 0),
                                stop=last or stage1_here,
                                skip_group_check=(split2 and kt == nkt - 1),
                            )
                            if stage1_here:
                                uniq = f"{s0}_{q0}_{h}_{m}_s1"
                                t65s = opool.tile(
                                    [65, 512], f32, tag="t65", name=f"t65_{uniq}"
                                )
                                nc.vector.tensor_copy(
                                    t65s[:, :c_split], oT_ps[m][:, :c_split]
                                )
                                other = m - 1 if m >= 1 else m + 1
                                finalize_range(
                                    s0, q0, h, m, edges[m], 0, c_split,
                                    t65s, f"oT_{other}", False, uniq,
                                )
                                tail_t65 = t65s
                            if last:
                                if split2:
                                    uniq = f"{s0}_{q0}_{h}_{m}_s2"
                                    nc.scalar.copy(
                                        tail_t65[:, c_split:blen],
                                        oT_ps[m][:, c_split:blen],
                                    )
                                    finalize_range(
                                        s0, q0, h, m, edges[m], c_split, blen,
                                        tail_t65, f"oT_{m}", True, uniq,
                                    )
                                    continue
                                args = (s0, q0, h, m, edges[m], blen, oT_ps[m])
                                if m == nblk - 1:
                                    # closes at pass end: defer past the next
                                    # pass's first QK
                                    pending_final = args
                                else:
                                    finalize_block(*args)

        if pending_final is not None:
            finalize_block(*pending_final, act_copy=True)
            pending_final = None

    nc.compile()
    return nc


def _segments_from_cu(cu_seqlens, T):
    edges = sorted(set([0, T] + [int(c) for c in cu_seqlens if 0 < int(c) < T]))
    return [(edges[i], edges[i + 1]) for i in range(len(edges) - 1)]


def kernel(q, k, v, k_cache, v_cache, slot_mapping, cu_seqlens):
    global LAST_RESULT
    T = q.shape[0]
    nslots = k_cache.shape[0]

    # Emulate scatter-then-gather through the paged cache: for duplicate slots
    # the last writer wins, so token i reads back k[lastw[slot[i]]].
    slot = np.asarray(slot_mapping, dtype=np.int64)
    lastw = np.zeros(nslots, dtype=np.int64)
    lastw[slot] = np.arange(T)
    lw = lastw[slot]
    k_eff = np.asarray(k)[lw]
    v_eff = np.asarray(v)[lw]

    segments = _segments_from_cu(np.asarray(cu_seqlens), T)
    key = (T, tuple(segments))
    if key not in _PROGRAM_CACHE:
        _PROGRAM_CACHE[key] = _build_program(T, segments)
    nc = _PROGRAM_CACHE[key]

    bf = ml_dtypes.bfloat16
    qh = np.ascontiguousarray(
        np.asarray(q).reshape(T, NKV * G, D).transpose(1, 2, 0)
    ).astype(bf)  # [16, 64, T]
    kh = np.ascontiguousarray(k_eff.reshape(T, NKV, D).transpose(1, 2, 0)).astype(bf)
    vh = v_eff.reshape(T, NKV, D).astype(bf)  # [T, 8, 64]

    in_maps = [
        {
            "qT": np.ascontiguousarray(qh[2 * h : 2 * h + 2].reshape(128, T)),
            "kT": np.ascontiguousarray(kh[h]),
            "v": np.ascontiguousarray(vh[:, h, :]),
        }
        for h in range(NKV)
    ]

    res = run_bass_kernel_spmd(nc, in_maps, core_ids=list(range(8)), trace=TRACE)
    LAST_RESULT = res

    out = np.empty((T, NKV * G * D), dtype=np.float32)
    ov = out.reshape(T, NKV, G * D)
    for h in range(NKV):
        ov[:, h, :] = res.results[h]["o"]
    return out
